# revision 17
# baseline (speedup 1.0000x reference)
"""Distributed transformer block (B=2, T=2048, C=1024, H=16) on 8 trn2 cores.

Sharding: heads for attention (2 heads/core), tokens for LN/FFN (512 tok/core).

Exchange structure (all contiguous-chunk DMAs, no rearrange scatters):
  A2A#1: k (pre-biased, [d2,t]) + v (pre-biased AND pre-transposed to [t,d2])
         fused in one 2MB AllToAll, triggered right after the k+v matmul
         groups -- overlaps the q matmul group.
  A2A#2: q (pre-biased), 1MB, queued behind #1 on the CC engine.
  A2A#3: attention output, packed incrementally per (batch, q-block).
QKV biases (with LN affine folds) are added pre-exchange as per-partition
scalars, so no post-exchange bias pass is needed.

Reference semantics quirk: softmax over the QUERY axis (axis=2 of bhqk).
Scores are computed in [k, q] layout so normalization is a free-axis
reduction; causal mask via affine_select after exp; 1/rowsum folded into v.
"""

import numpy as np
import ml_dtypes

import concourse.bass as bass
import concourse.mybir as mybir
import concourse.tile as tile
from concourse.bass_utils import run_bass_kernel_spmd
from concourse.masks import make_identity

# problem shapes (hardcoded per harness contract)
B, T, C, H = 2, 2048, 1024, 16
HS = C // H          # 64
EPS = 1e-5
NC_ = 8              # cores
TSH = B * T // NC_   # 512 tokens per core
HPC = H // NC_       # 2 heads per core
D2 = HPC * HS        # 128 (2 heads side by side)
P = 128
F32 = mybir.dt.float32
BF16 = mybir.dt.bfloat16

KT = T // P          # 16 k-tiles per batch
QT = T // 512        # 4 q-tiles of 512 per batch
CO = C // P          # 8 c-chunks


def split_waits(nc, max_waits=1):
    """This container's walrus rejects >1 sem-wait per instruction; move
    excess waits onto preceding same-engine NOPs."""
    n = 0
    for bb in nc.main_func.blocks:
        new_insts = []
        for ins in bb.instructions:
            si = ins.sync_info
            if si is not None and si.on_wait and len(si.on_wait) > max_waits:
                waits = list(si.on_wait)
                keep = waits[:max_waits]
                extra = waits[max_waits:]
                chunks = [extra[i:i + max_waits] for i in range(0, len(extra), max_waits)]
                for ci, chunk in enumerate(chunks):
                    new_insts.append(mybir.InstNoOp(
                        name=f"{ins.name}-waitnop{ci}",
                        engine=ins.engine,
                        sync_info=mybir.SyncInfo(on_wait=list(chunk), on_update=[]),
                        text_hint="split_waits",
                    ))
                si.on_wait = keep
                n += 1
            new_insts.append(ins)
        bb.instructions[:] = new_insts
    return n


def _ln_apply(nc, pool, x_view, out_view, eps_t):
    """LayerNorm (no affine: folded into weights): out = (x - m) * rsqrt(var+eps).
    x_view/out_view: [128, 1024]."""
    stats = pool.tile([P, 2, 6], F32, tag="ln_stats")
    nc.vector.bn_stats(out=stats[:, 0, :], in_=x_view[:, 0:512])
    nc.vector.bn_stats(out=stats[:, 1, :], in_=x_view[:, 512:1024])
    mv = pool.tile([P, 2], F32, tag="ln_mv")
    nc.vector.bn_aggr(out=mv, in_=stats)
    nc.scalar.activation(out=mv[:, 1:2], in_=mv[:, 1:2],
                         func=mybir.ActivationFunctionType.Sqrt,
                         bias=eps_t, scale=1.0)
    nc.vector.reciprocal(out=mv[:, 1:2], in_=mv[:, 1:2])
    nc.vector.tensor_scalar(out=out_view, in0=x_view,
                            scalar1=mv[:, 0:1], scalar2=mv[:, 1:2],
                            op0=mybir.AluOpType.subtract,
                            op1=mybir.AluOpType.mult)


def build_nc():
    nc = bass.Bass(num_devices=NC_, num_swdge_queues=4)

    # ---- per-core external I/O ----
    x_sh = nc.dram_tensor("x_sh", [TSH, C], F32, kind="ExternalInput")
    wk = nc.dram_tensor("wk", [C, C], BF16, kind="ExternalInput")
    wq = nc.dram_tensor("wq", [C, C], BF16, kind="ExternalInput")
    wv = nc.dram_tensor("wv", [C, C], BF16, kind="ExternalInput")
    bqkv = nc.dram_tensor("bqkv", [3, C], F32, kind="ExternalInput")
    wo = nc.dram_tensor("wo", [C, C], BF16, kind="ExternalInput")
    bo = nc.dram_tensor("bo", [C], F32, kind="ExternalInput")
    w1 = nc.dram_tensor("w1", [C, C], BF16, kind="ExternalInput")
    bf1 = nc.dram_tensor("bf1", [C], F32, kind="ExternalInput")
    w2 = nc.dram_tensor("w2", [C, C], BF16, kind="ExternalInput")
    bf2 = nc.dram_tensor("bf2", [C], F32, kind="ExternalInput")
    out_sh = nc.dram_tensor("out_sh", [TSH, C], F32, kind="ExternalOutput")

    rg = [list(range(NC_))]

    with tile.TileContext(nc) as tc:
        with tc.tile_pool(name="persist", bufs=1) as pp, \
             tc.tile_pool(name="dram", bufs=1, space="DRAM") as dp:
            wp = tc.alloc_tile_pool(name="wpool", bufs=1)

            # ---------- weight/x loads (issue order = use order) ----------
            x_all = pp.tile([P, 4, C], F32)   # own 512 tokens, [t_i, t_o, c]
            for ti in range(4):
                nc.sync.dma_start(x_all[:, ti, :], x_sh[ti * P:(ti + 1) * P, :])
            wk_sb = wp.tile([P, CO, C], BF16)
            nc.sync.dma_start(wk_sb, wk.rearrange("(o i) n -> i o n", i=P))
            wq_sb = wp.tile([P, CO, C], BF16)
            nc.sync.dma_start(wq_sb, wq.rearrange("(o i) n -> i o n", i=P))
            wv_sb = wp.tile([P, CO, C], BF16)
            nc.sync.dma_start(wv_sb, wv.rearrange("(o i) n -> i o n", i=P))

            eps_t = pp.tile([P, 1], F32)
            nc.vector.memset(eps_t, EPS)
            ident = pp.tile([P, P], F32)
            make_identity(nc, ident)
            ident_bf = pp.tile([P, P], BF16)
            nc.vector.tensor_copy(out=ident_bf, in_=ident)

            # per-(qkv, rank) bias columns: [d2_i, qkv, rank]
            bqkv_sb = pp.tile([P, 3, NC_], F32)
            nc.sync.dma_start(bqkv_sb, bqkv.rearrange("q (r d) -> d q r", d=P))
            bf1_sb = pp.tile([P, CO], F32)
            nc.sync.dma_start(bf1_sb, bf1.rearrange("(o i) -> i o", i=P))
            bo_bc = pp.tile([P, C], F32)
            nc.gpsimd.dma_start(bo_bc, bo[:].partition_broadcast(P))
            bf2_bc = pp.tile([P, C], F32)
            nc.gpsimd.dma_start(bf2_bc, bf2[:].partition_broadcast(P))

            # late-phase weights: prefetch now, consumed after attention
            wo_sb = pp.tile([P, CO, C], BF16)
            nc.sync.dma_start(wo_sb, wo.rearrange("(o i) n -> i o n", i=P))
            w1_sb = pp.tile([P, CO, C], BF16)
            nc.sync.dma_start(w1_sb, w1.rearrange("(o i) n -> i o n", i=P))
            w2_sb = pp.tile([P, CO, C], BF16)
            nc.sync.dma_start(w2_sb, w2.rearrange("(o i) n -> i o n", i=P))

            x2_all = pp.tile([P, 4, C], F32)  # post-attention residual state

            # A2A buffers (contiguous per-rank chunks)
            a2a1_in = dp.tile([NC_, 2, P, TSH], BF16)               # k | q
            a2a1_out = dp.tile([NC_, 2, P, TSH], BF16)
            a2a2_in = dp.tile([NC_, P, TSH], BF16)                  # vT
            a2a2_out = dp.tile([NC_, P, TSH], BF16)
            a2a3_in = dp.tile([NC_, P, TSH], BF16)                  # att
            a2a3_out = dp.tile([NC_, P, TSH], BF16)

            # ---------- P1: LN1 + transpose own shard ----------
            sc_p1 = nc.enter_named_scope("P1_ln1", False)
            with tc.tile_pool(name="p1w", bufs=4) as p1w, \
                 tc.tile_pool(name="hT_pool", bufs=1) as hp, \
                 tc.tile_pool(name="ps_tr", bufs=2, space="PSUM") as ptr:
                hT_sb = hp.tile([P, CO, TSH], BF16)  # [c_i, c_o, t_local]
                h_ts = []
                for ti in range(4):
                    h_t = p1w.tile([P, C], F32, tag=f"h_t{ti}",
                                   name=f"h_t{ti}", bufs=1)
                    _ln_apply(nc, p1w, x_all[:, ti, :], h_t, eps_t)
                    h_ts.append(h_t)
                for cj in range(CO):
                    for ti in range(4):
                        ps = ptr.tile([P, P], F32, tag="tr")
                        nc.tensor.transpose(
                            ps, h_ts[ti][:, cj * P:(cj + 1) * P], ident)
                        nc.vector.tensor_copy(
                            out=hT_sb[:, cj, ti * P:(ti + 1) * P], in_=ps)
                nc.leave_named_scope("P1_ln1", sc_p1[0], False)

                # ---------- P2: QKV for ALL heads over OWN tokens ----------
                # group order k, q, v: A2A#1 (k+q) launches before v computes,
                # so scores can start while v's A2A#2 is still in flight.
                sc_p2 = nc.enter_named_scope("P2_qkv", False)
                with tc.tile_pool(name="stg", bufs=4) as stg, \
                     tc.tile_pool(name="ps_qkv", bufs=4, space="PSUM") as pq, \
                     tc.tile_pool(name="ps_vtr", bufs=2, space="PSUM") as pv:
                    for qkv, w_sb in ((1, wk_sb), (0, wq_sb), (2, wv_sb)):
                        for r in range(NC_):
                            psd = pq.tile([P, TSH], F32, tag="psd")
                            for cj in range(CO):
                                nc.tensor.matmul(
                                    psd, w_sb[:, cj, r * P:(r + 1) * P],
                                    hT_sb[:, cj, :],
                                    start=(cj == 0), stop=(cj == CO - 1))
                            st = stg.tile([P, TSH], BF16, tag="st")
                            nc.scalar.add(st, psd, bqkv_sb[:, qkv, r:r + 1])
                            if qkv == 1:    # k: ship [d2, t]
                                nc.sync.dma_start(a2a1_in[r, 0], st)
                            elif qkv == 0:  # q: ship [d2, t]
                                nc.sync.dma_start(a2a1_in[r, 1], st)
                            else:           # v: pre-transpose to [t, d2]
                                vtr = stg.tile([P, 4, P], BF16, tag="vtr")
                                for c4 in range(4):
                                    ps2 = pv.tile([P, P], BF16, tag="vt")
                                    nc.tensor.transpose(
                                        ps2, st[:, c4 * P:(c4 + 1) * P],
                                        ident_bf)
                                    nc.vector.tensor_copy(
                                        out=vtr[:, c4, :], in_=ps2)
                                nc.sync.dma_start(a2a2_in[r],
                                                  vtr.rearrange("p c d -> p (c d)"))
                        if qkv == 0:  # k and q staged -> launch A2A#1
                            nc.gpsimd.collective_compute(
                                "AllToAll", mybir.AluOpType.bypass,
                                ins=[a2a1_in.opt()], outs=[a2a1_out.opt()],
                                replica_groups=rg)
                    nc.gpsimd.collective_compute(
                        "AllToAll", mybir.AluOpType.bypass,
                        ins=[a2a2_in.opt()], outs=[a2a2_out.opt()],
                        replica_groups=rg)
                nc.leave_named_scope("P2_qkv", sc_p2[0], False)
            wp.release()

            # ---------- P3+P4 shared SBUF: unpack + attention ----------
            with tc.tile_pool(name="pqkv", bufs=1) as pqk:
                kT = pqk.tile([P, B * T], BF16)           # [d2, t_glob]
                qT = pqk.tile([P, B * T], BF16)
                v_sb = pqk.tile([P, B * KT, D2], BF16)    # [k_i, k_chunk, d2]

                for s in range(NC_):
                    nc.sync.dma_start(kT[:, s * TSH:(s + 1) * TSH],
                                      a2a1_out[s, 0])
                    nc.sync.dma_start(qT[:, s * TSH:(s + 1) * TSH],
                                      a2a1_out[s, 1])
                for s in range(NC_):
                    nc.sync.dma_start(
                        v_sb[:, s * 4:(s + 1) * 4, :],
                        a2a2_out[s].rearrange("p (c d) -> p c d", d=P))

                # ---------- P4: attention (per head, both batches) ----------
                sc_p4 = nc.enter_named_scope("P4_attn", False)
                # scores in [k, q] layout; per-j PSUM score tiles; heads share
                # att accumulator banks (h0 -> partitions 0:64, h1 -> 64:128).
                with tc.tile_pool(name="p4w", bufs=4) as p4w, \
                     tc.tile_pool(name="p4o", bufs=3) as p4o, \
                     tc.tile_pool(name="ps_att", bufs=1, space="PSUM") as pa, \
                     tc.tile_pool(name="ps_sc", bufs=4, space="PSUM") as psc:
                    for b in range(B):
                        att_ps = [pa.tile([P, 512], F32, tag=f"att{j}",
                                          name=f"att_ps{j}")
                                  for j in range(QT)]
                        for kt in range(KT):
                            k0 = kt * P
                            jmin = k0 // 512
                            o = k0 - 512 * jmin
                            for h2 in range(2):
                                hsl = slice(h2 * HS, (h2 + 1) * HS)
                                wTe = p4w.tile([P, T], BF16, tag="wTe",
                                               bufs=6)
                                s_part = p4w.tile([P, QT], F32, tag="s_part")
                                rs = p4w.tile([P, 1], F32, tag="rs")
                                for j in range(jmin, QT):
                                    if j == jmin:
                                        c0 = j * 512 + min(o, 256)
                                    else:
                                        c0 = j * 512
                                    w = (j + 1) * 512 - c0
                                    ps = psc.tile([P, 512], F32, tag="sc")
                                    nc.tensor.matmul(
                                        ps[:, 0:w],
                                        kT[hsl, b * T + k0:b * T + k0 + P],
                                        qT[hsl, b * T + c0:b * T + (j + 1) * 512],
                                        start=True, stop=True)
                                    if j == jmin:
                                        vs = k0 - c0
                                        nc.scalar.activation(
                                            out=wTe[:, k0:(j + 1) * 512],
                                            in_=ps[:, vs:w],
                                            func=mybir.ActivationFunctionType.Exp)
                                        nc.gpsimd.affine_select(
                                            out=wTe[:, k0:k0 + P],
                                            in_=wTe[:, k0:k0 + P],
                                            compare_op=mybir.AluOpType.is_ge,
                                            fill=0.0, base=0,
                                            pattern=[[1, P]], channel_multiplier=-1)
                                        nc.vector.reduce_sum(
                                            out=s_part[:, j:j + 1],
                                            in_=wTe[:, k0:(j + 1) * 512],
                                            axis=mybir.AxisListType.X)
                                    else:
                                        nc.scalar.activation(
                                            out=wTe[:, j * 512:(j + 1) * 512],
                                            in_=ps[:, 0:512],
                                            func=mybir.ActivationFunctionType.Exp,
                                            accum_out=s_part[:, j:j + 1])
                                nc.vector.reduce_sum(out=rs,
                                                     in_=s_part[:, jmin:QT],
                                                     axis=mybir.AxisListType.X)
                                nc.vector.reciprocal(out=rs, in_=rs)
                                vp = p4w.tile([P, HS], BF16, tag="vp")
                                nc.vector.tensor_scalar_mul(
                                    out=vp, in0=v_sb[:, b * KT + kt, hsl],
                                    scalar1=rs)
                                for j in range(jmin, QT):
                                    c0 = j * 512 + (o if j == jmin else 0)
                                    nc.tensor.matmul(
                                        att_ps[j][h2 * HS:(h2 + 1) * HS,
                                                  c0 - j * 512:512],
                                        vp, wTe[:, c0:(j + 1) * 512],
                                        start=(kt == 0), stop=(kt == 4 * j + 3),
                                        tile_position=(0, h2 * HS))
                        for j in range(QT):
                            oc = p4o.tile([P, 512], BF16, tag="oc")
                            nc.vector.tensor_copy(out=oc, in_=att_ps[j])
                            nc.sync.dma_start(a2a3_in[b * QT + j], oc)
                nc.leave_named_scope("P4_attn", sc_p4[0], False)
            nc.gpsimd.collective_compute(
                "AllToAll", mybir.AluOpType.bypass,
                ins=[a2a3_in.opt()], outs=[a2a3_out.opt()], replica_groups=rg)

            # ---------- P6: Wo + residual ----------
            sc_p6 = nc.enter_named_scope("P6_wo", False)
            with tc.tile_pool(name="p6", bufs=1) as p6, \
                 tc.tile_pool(name="pffn", bufs=1) as pf, \
                 tc.tile_pool(name="ps_wo", bufs=2, space="PSUM") as pw, \
                 tc.tile_pool(name="ps_tr2", bufs=2, space="PSUM") as ptr, \
                 tc.tile_pool(name="ps_z", bufs=2, space="PSUM") as pz, \
                 tc.tile_pool(name="ps_y", bufs=2, space="PSUM") as py, \
                 tc.tile_pool(name="p7w", bufs=2) as p7w, \
                 tc.tile_pool(name="p9w", bufs=2) as p9w:
                # attTs: [c_in_i=128, c_in_chunk=8, t_local=512]
                attTs = p6.tile([P, NC_, TSH], BF16)
                for s in range(NC_):
                    nc.sync.dma_start(attTs[:, s, :], a2a3_out[s])
                for ti in range(4):
                    for cj in range(2):
                        ps = pw.tile([P, 512], F32, tag="wo")
                        for r in range(NC_):
                            nc.tensor.matmul(
                                ps,
                                attTs[:, r, ti * P:(ti + 1) * P],
                                wo_sb[:, r, cj * 512:(cj + 1) * 512],
                                start=(r == 0), stop=(r == NC_ - 1))
                        csl = slice(cj * 512, (cj + 1) * 512)
                        nc.vector.tensor_add(out=x2_all[:, ti, csl], in0=ps,
                                             in1=x_all[:, ti, csl])
                        nc.vector.tensor_add(out=x2_all[:, ti, csl],
                                             in0=x2_all[:, ti, csl],
                                             in1=bo_bc[:, csl])
                nc.leave_named_scope("P6_wo", sc_p6[0], False)

                # ---------- P7-P9 (same pool scope for overlap) ----------
                h2T_sb = pf.tile([P, CO, TSH], BF16)
                uT_sb = pf.tile([P, CO, TSH], BF16)  # [j_i, j_o, t]

                # P7: LN2 + transpose
                sc_p7 = nc.enter_named_scope("P7_ln2", False)
                for ti in range(4):
                    h2_t = p7w.tile([P, C], F32, tag="h2_t")
                    _ln_apply(nc, p7w, x2_all[:, ti, :], h2_t, eps_t)
                    for cj in range(CO):
                        ps = ptr.tile([P, P], F32, tag="tr2")
                        nc.tensor.transpose(ps, h2_t[:, cj * P:(cj + 1) * P],
                                            ident)
                        nc.vector.tensor_copy(
                            out=h2T_sb[:, cj, ti * P:(ti + 1) * P], in_=ps)
                nc.leave_named_scope("P7_ln2", sc_p7[0], False)

                # P8: FFN1 (zT = W1'^T h2T, relu+bias)
                sc_p8 = nc.enter_named_scope("P8_ffn1", False)
                for jt in range(CO):
                    ps = pz.tile([P, TSH], F32, tag="z")
                    for cj in range(CO):
                        nc.tensor.matmul(
                            ps, w1_sb[:, cj, jt * P:(jt + 1) * P],
                            h2T_sb[:, cj, :],
                            start=(cj == 0), stop=(cj == CO - 1))
                    nc.scalar.activation(
                        out=uT_sb[:, jt, :], in_=ps,
                        func=mybir.ActivationFunctionType.Relu,
                        bias=bf1_sb[:, jt:jt + 1], scale=1.0)
                nc.leave_named_scope("P8_ffn1", sc_p8[0], False)

                # P9: FFN2 + residual -> out
                sc_p9 = nc.enter_named_scope("P9_ffn2", False)
                for ti in range(4):
                    for cj in range(2):
                        ps = py.tile([P, 512], F32, tag="y")
                        for jc in range(CO):
                            nc.tensor.matmul(
                                ps, uT_sb[:, jc, ti * P:(ti + 1) * P],
                                w2_sb[:, jc, cj * 512:(cj + 1) * 512],
                                start=(jc == 0), stop=(jc == CO - 1))
                        csl = slice(cj * 512, (cj + 1) * 512)
                        o_t = p9w.tile([P, 512], F32, tag="o_t")
                        nc.vector.tensor_add(out=o_t, in0=ps,
                                             in1=x2_all[:, ti, csl])
                        nc.vector.tensor_add(out=o_t, in0=o_t,
                                             in1=bf2_bc[:, csl])
                        nc.sync.dma_start(
                            out_sh[ti * P:(ti + 1) * P, csl], o_t)
                nc.leave_named_scope("P9_ffn2", sc_p9[0], False)

    split_waits(nc)
    return nc


_NC_CACHE = None


def _get_nc():
    global _NC_CACHE
    if _NC_CACHE is None:
        _NC_CACHE = build_nc()
    return _NC_CACHE


def _prep_inputs(inputs):
    """Host-side weight folding + per-core sharding."""
    x = np.asarray(inputs["x"], np.float32)
    Wq, bq = np.asarray(inputs["Wq"], np.float32), np.asarray(inputs["bq"], np.float32)
    Wk, bk = np.asarray(inputs["Wk"], np.float32), np.asarray(inputs["bk"], np.float32)
    Wv, bv = np.asarray(inputs["Wv"], np.float32), np.asarray(inputs["bv"], np.float32)
    Wo, bo = np.asarray(inputs["Wo"], np.float32), np.asarray(inputs["bo"], np.float32)
    g1, b1 = np.asarray(inputs["g1"], np.float32), np.asarray(inputs["b1"], np.float32)
    g2, b2 = np.asarray(inputs["g2"], np.float32), np.asarray(inputs["b2"], np.float32)
    W1, bf1 = np.asarray(inputs["W1"], np.float32), np.asarray(inputs["bf1"], np.float32)
    W2, bf2 = np.asarray(inputs["W2"], np.float32), np.asarray(inputs["bf2"], np.float32)

    scale = float(HS) ** -0.5
    xf = x.reshape(B * T, C)
    # folded FFN1: h2@W1+bf1 with h2 = ln*g2+b2 -> ln @ (g2*W1) + (b2@W1+bf1)
    w1f = (g2[:, None] * W1).astype(np.float32)
    bf1f = (b2 @ W1 + bf1).astype(np.float32)

    # per-qkv all-head weights [C, C], columns rank-major (rank, head, hs)
    Wq_f = (g1[:, None, None] * Wq.transpose(1, 0, 2).reshape(C, H, HS)
            ).reshape(C, C) * scale
    Wk_f = (g1[:, None, None] * Wk.transpose(1, 0, 2).reshape(C, H, HS)
            ).reshape(C, C)
    Wv_f = (g1[:, None, None] * Wv.transpose(1, 0, 2).reshape(C, H, HS)
            ).reshape(C, C)
    bq_f = (b1 @ Wq.transpose(1, 0, 2).reshape(C, C)
            + bq.reshape(C)) * scale
    bk_f = b1 @ Wk.transpose(1, 0, 2).reshape(C, C) + bk.reshape(C)
    bv_f = b1 @ Wv.transpose(1, 0, 2).reshape(C, C) + bv.reshape(C)
    bqkv_all = np.ascontiguousarray(
        np.stack([bq_f, bk_f, bv_f]).astype(np.float32))

    common = {
        "wk": np.ascontiguousarray(Wk_f.astype(ml_dtypes.bfloat16)),
        "wq": np.ascontiguousarray(Wq_f.astype(ml_dtypes.bfloat16)),
        "wv": np.ascontiguousarray(Wv_f.astype(ml_dtypes.bfloat16)),
        "bqkv": bqkv_all,
        "wo": np.ascontiguousarray(Wo.astype(ml_dtypes.bfloat16)),
        "bo": np.ascontiguousarray(bo),
        "w1": np.ascontiguousarray(w1f.astype(ml_dtypes.bfloat16)),
        "bf1": np.ascontiguousarray(bf1f),
        "w2": np.ascontiguousarray(W2.astype(ml_dtypes.bfloat16)),
        "bf2": np.ascontiguousarray(bf2),
    }
    in_maps = []
    for r in range(NC_):
        m = dict(common)
        m["x_sh"] = np.ascontiguousarray(xf[r * TSH:(r + 1) * TSH])
        in_maps.append(m)
    return in_maps


def run(inputs, trace=False):
    nc = _get_nc()
    in_maps = _prep_inputs(inputs)
    res = run_bass_kernel_spmd(nc, in_maps, core_ids=list(range(NC_)), trace=trace)
    out = np.concatenate([res.results[r]["out_sh"] for r in range(NC_)], axis=0)
    return out.reshape(B, T, C), res


def kernel(**inputs) -> np.ndarray:
    out, _ = run(inputs, trace=False)
    return out


# revision 21
# speedup vs baseline: 1.2539x; 1.2539x over previous
"""Distributed transformer block (B=2, T=2048, C=1024, H=16) on 8 trn2 cores.

Sharding: heads for attention (2 heads/core), tokens for LN/FFN (512 tok/core).

Exchange structure (all contiguous-chunk DMAs, no rearrange scatters):
  A2A#1: k (pre-biased, [d2,t]) + v (pre-biased AND pre-transposed to [t,d2])
         fused in one 2MB AllToAll, triggered right after the k+v matmul
         groups -- overlaps the q matmul group.
  A2A#2: q (pre-biased), 1MB, queued behind #1 on the CC engine.
  A2A#3: attention output, packed incrementally per (batch, q-block).
QKV biases (with LN affine folds) are added pre-exchange as per-partition
scalars, so no post-exchange bias pass is needed.

Reference semantics quirk: softmax over the QUERY axis (axis=2 of bhqk).
Scores are computed in [k, q] layout so normalization is a free-axis
reduction; causal mask via affine_select after exp; 1/rowsum folded into v.
"""

import numpy as np
import ml_dtypes

import concourse.bass as bass
import concourse.mybir as mybir
import concourse.tile as tile
from concourse.bass_utils import run_bass_kernel_spmd
from concourse.masks import make_identity

# problem shapes (hardcoded per harness contract)
B, T, C, H = 2, 2048, 1024, 16
HS = C // H          # 64
EPS = 1e-5
NC_ = 8              # cores
TSH = B * T // NC_   # 512 tokens per core
HPC = H // NC_       # 2 heads per core
D2 = HPC * HS        # 128 (2 heads side by side)
P = 128
F32 = mybir.dt.float32
BF16 = mybir.dt.bfloat16

KT = T // P          # 16 k-tiles per batch
QT = T // 512        # 4 q-tiles of 512 per batch
CO = C // P          # 8 c-chunks


def split_waits(nc, max_waits=1):
    """This container's walrus rejects >1 sem-wait per instruction; move
    excess waits onto preceding same-engine NOPs."""
    n = 0
    for bb in nc.main_func.blocks:
        new_insts = []
        for ins in bb.instructions:
            si = ins.sync_info
            if si is not None and si.on_wait and len(si.on_wait) > max_waits:
                waits = list(si.on_wait)
                keep = waits[:max_waits]
                extra = waits[max_waits:]
                chunks = [extra[i:i + max_waits] for i in range(0, len(extra), max_waits)]
                for ci, chunk in enumerate(chunks):
                    new_insts.append(mybir.InstNoOp(
                        name=f"{ins.name}-waitnop{ci}",
                        engine=ins.engine,
                        sync_info=mybir.SyncInfo(on_wait=list(chunk), on_update=[]),
                        text_hint="split_waits",
                    ))
                si.on_wait = keep
                n += 1
            new_insts.append(ins)
        bb.instructions[:] = new_insts
    return n


def _ln_apply(nc, pool, x_view, out_view, eps_t):
    """LayerNorm (no affine: folded into weights): out = (x - m) * rsqrt(var+eps).
    x_view/out_view: [128, 1024]."""
    stats = pool.tile([P, 2, 6], F32, tag="ln_stats")
    nc.vector.bn_stats(out=stats[:, 0, :], in_=x_view[:, 0:512])
    nc.vector.bn_stats(out=stats[:, 1, :], in_=x_view[:, 512:1024])
    mv = pool.tile([P, 2], F32, tag="ln_mv")
    nc.vector.bn_aggr(out=mv, in_=stats)
    nc.scalar.activation(out=mv[:, 1:2], in_=mv[:, 1:2],
                         func=mybir.ActivationFunctionType.Sqrt,
                         bias=eps_t, scale=1.0)
    nc.vector.reciprocal(out=mv[:, 1:2], in_=mv[:, 1:2])
    nc.vector.tensor_scalar(out=out_view, in0=x_view,
                            scalar1=mv[:, 0:1], scalar2=mv[:, 1:2],
                            op0=mybir.AluOpType.subtract,
                            op1=mybir.AluOpType.mult)


def build_nc():
    nc = bass.Bass(num_devices=NC_, num_swdge_queues=4)

    # ---- per-core external I/O ----
    x_sh = nc.dram_tensor("x_sh", [TSH, C], F32, kind="ExternalInput")
    wk = nc.dram_tensor("wk", [C, C], BF16, kind="ExternalInput")
    wq = nc.dram_tensor("wq", [C, C], BF16, kind="ExternalInput")
    wv = nc.dram_tensor("wv", [C, C], BF16, kind="ExternalInput")
    bqkv = nc.dram_tensor("bqkv", [3, C], F32, kind="ExternalInput")
    wo = nc.dram_tensor("wo", [C, C], BF16, kind="ExternalInput")
    bo = nc.dram_tensor("bo", [C], F32, kind="ExternalInput")
    w1 = nc.dram_tensor("w1", [C, C], BF16, kind="ExternalInput")
    bf1 = nc.dram_tensor("bf1", [C], F32, kind="ExternalInput")
    w2 = nc.dram_tensor("w2", [C, C], BF16, kind="ExternalInput")
    bf2 = nc.dram_tensor("bf2", [C], F32, kind="ExternalInput")
    out_sh = nc.dram_tensor("out_sh", [TSH, C], F32, kind="ExternalOutput")

    rg = [list(range(NC_))]

    with tile.TileContext(nc) as tc:
        with tc.tile_pool(name="persist", bufs=1) as pp, \
             tc.tile_pool(name="dram", bufs=1, space="DRAM") as dp:
            wp = tc.alloc_tile_pool(name="wpool", bufs=1)

            # ---------- weight/x loads (issue order = use order) ----------
            x_all = pp.tile([P, 4, C], F32)   # own 512 tokens, [t_i, t_o, c]
            for ti in range(4):
                nc.sync.dma_start(x_all[:, ti, :], x_sh[ti * P:(ti + 1) * P, :])
            wk_sb = wp.tile([P, CO, C], BF16)
            nc.sync.dma_start(wk_sb, wk.rearrange("(o i) n -> i o n", i=P))
            wq_sb = wp.tile([P, CO, C], BF16)
            nc.sync.dma_start(wq_sb, wq.rearrange("(o i) n -> i o n", i=P))
            wv_sb = wp.tile([P, CO, C], BF16)
            nc.sync.dma_start(wv_sb, wv.rearrange("(o i) n -> i o n", i=P))

            eps_t = pp.tile([P, 1], F32)
            nc.vector.memset(eps_t, EPS)
            ident = pp.tile([P, P], F32)
            make_identity(nc, ident)
            ident_bf = pp.tile([P, P], BF16)
            nc.vector.tensor_copy(out=ident_bf, in_=ident)

            # per-(qkv, rank) bias columns: [d2_i, qkv, rank]
            bqkv_sb = pp.tile([P, 3, NC_], F32)
            nc.sync.dma_start(bqkv_sb, bqkv.rearrange("q (r d) -> d q r", d=P))
            bf1_sb = pp.tile([P, CO], F32)
            nc.sync.dma_start(bf1_sb, bf1.rearrange("(o i) -> i o", i=P))
            bo_bc = pp.tile([P, C], F32)
            nc.gpsimd.dma_start(bo_bc, bo[:].partition_broadcast(P))
            bf2_bc = pp.tile([P, C], F32)
            nc.gpsimd.dma_start(bf2_bc, bf2[:].partition_broadcast(P))

            # late-phase weights: prefetch now, consumed after attention
            wo_sb = pp.tile([P, CO, C], BF16)
            nc.sync.dma_start(wo_sb, wo.rearrange("(o i) n -> i o n", i=P))
            w1_sb = pp.tile([P, CO, C], BF16)
            nc.sync.dma_start(w1_sb, w1.rearrange("(o i) n -> i o n", i=P))
            w2_sb = pp.tile([P, CO, C], BF16)
            nc.sync.dma_start(w2_sb, w2.rearrange("(o i) n -> i o n", i=P))

            x2_all = pp.tile([P, 4, C], F32)  # post-attention residual state

            # A2A buffers (contiguous per-rank chunks)
            a2a1_in = dp.tile([NC_, 2, P, TSH], BF16)               # k | vT
            a2a1_out = dp.tile([NC_, 2, P, TSH], BF16)
            a2a2_in = dp.tile([NC_, P, TSH], BF16)                  # q
            a2a2_out = dp.tile([NC_, P, TSH], BF16)
            a2a3_in = dp.tile([NC_, P, TSH], BF16)                  # att
            a2a3_out = dp.tile([NC_, P, TSH], BF16)

            # ---------- P1: LN1 + transpose own shard ----------
            sc_p1 = nc.enter_named_scope("P1_ln1", False)
            with tc.tile_pool(name="p1w", bufs=4) as p1w, \
                 tc.tile_pool(name="hT_pool", bufs=1) as hp, \
                 tc.tile_pool(name="ps_tr", bufs=2, space="PSUM") as ptr:
                hT_sb = hp.tile([P, CO, TSH], BF16)  # [c_i, c_o, t_local]
                h_ts = []
                for ti in range(4):
                    h_t = p1w.tile([P, C], F32, tag=f"h_t{ti}",
                                   name=f"h_t{ti}", bufs=1)
                    _ln_apply(nc, p1w, x_all[:, ti, :], h_t, eps_t)
                    h_ts.append(h_t)
                for cj in range(CO):
                    for ti in range(4):
                        ps = ptr.tile([P, P], F32, tag="tr")
                        nc.tensor.transpose(
                            ps, h_ts[ti][:, cj * P:(cj + 1) * P], ident)
                        nc.vector.tensor_copy(
                            out=hT_sb[:, cj, ti * P:(ti + 1) * P], in_=ps)
                nc.leave_named_scope("P1_ln1", sc_p1[0], False)

                # ---------- P2: QKV for ALL heads over OWN tokens ----------
                # group order k, v, q: A2A#1 (k+v) launches before q computes,
                # so only q's small A2A#2 gates the start of attention and v
                # is already resident when the first AV matmul needs it.
                sc_p2 = nc.enter_named_scope("P2_qkv", False)
                with tc.tile_pool(name="stg", bufs=4) as stg, \
                     tc.tile_pool(name="ps_qkv", bufs=4, space="PSUM") as pq, \
                     tc.tile_pool(name="ps_vtr", bufs=2, space="PSUM") as pv:
                    for qkv, w_sb in ((1, wk_sb), (2, wv_sb), (0, wq_sb)):
                        v_sts = []
                        for r in range(NC_):
                            psd = pq.tile([P, TSH], F32, tag="psd")
                            for cj in range(CO):
                                nc.tensor.matmul(
                                    psd, w_sb[:, cj, r * P:(r + 1) * P],
                                    hT_sb[:, cj, :],
                                    start=(cj == 0), stop=(cj == CO - 1))
                            if qkv == 1:    # k: ship [d2, t]
                                st = stg.tile([P, TSH], BF16, tag="st")
                                nc.scalar.add(st, psd,
                                              bqkv_sb[:, qkv, r:r + 1])
                                nc.sync.dma_start(a2a1_in[r, 0], st)
                            elif qkv == 0:  # q: ship [d2, t]
                                st = stg.tile([P, TSH], BF16, tag="st")
                                nc.scalar.add(st, psd,
                                              bqkv_sb[:, qkv, r:r + 1])
                                nc.sync.dma_start(a2a2_in[r], st)
                            else:           # v: stage all 8 first (transposes
                                # come after, so they never stall the PE FIFO)
                                st = stg.tile([P, TSH], BF16, tag="stv",
                                              bufs=NC_, name=f"stv{r}")
                                nc.scalar.add(st, psd,
                                              bqkv_sb[:, qkv, r:r + 1])
                                v_sts.append(st)
                        if qkv == 2:
                            for r in range(NC_):
                                vtr = stg.tile([P, 4, P], BF16, tag="vtr")
                                for c4 in range(4):
                                    ps2 = pv.tile([P, P], BF16, tag="vt")
                                    nc.tensor.transpose(
                                        ps2, v_sts[r][:, c4 * P:(c4 + 1) * P],
                                        ident_bf)
                                    nc.vector.tensor_copy(
                                        out=vtr[:, c4, :], in_=ps2)
                                nc.sync.dma_start(a2a1_in[r, 1],
                                                  vtr.rearrange("p c d -> p (c d)"))
                            # k and v staged -> launch A2A#1
                            nc.gpsimd.collective_compute(
                                "AllToAll", mybir.AluOpType.bypass,
                                ins=[a2a1_in.opt()], outs=[a2a1_out.opt()],
                                replica_groups=rg)
                    nc.gpsimd.collective_compute(
                        "AllToAll", mybir.AluOpType.bypass,
                        ins=[a2a2_in.opt()], outs=[a2a2_out.opt()],
                        replica_groups=rg)
                nc.leave_named_scope("P2_qkv", sc_p2[0], False)
            wp.release()

            # ---------- P3+P4 shared SBUF: unpack + attention ----------
            with tc.tile_pool(name="pqkv", bufs=1) as pqk:
                kT = pqk.tile([P, B * T], BF16)           # [d2, t_glob]
                qT = pqk.tile([P, B * T], BF16)
                v_sb = pqk.tile([P, B * KT, D2], BF16)    # [k_i, k_chunk, d2]

                for s in range(NC_):
                    nc.sync.dma_start(kT[:, s * TSH:(s + 1) * TSH],
                                      a2a1_out[s, 0])
                    nc.sync.dma_start(
                        v_sb[:, s * 4:(s + 1) * 4, :],
                        a2a1_out[s, 1].rearrange("p (c d) -> p c d", d=P))
                for s in range(NC_):
                    nc.sync.dma_start(qT[:, s * TSH:(s + 1) * TSH],
                                      a2a2_out[s])

                # ---------- P4: attention (per head, both batches) ----------
                sc_p4 = nc.enter_named_scope("P4_attn", False)
                # scores in [k, q] layout; per-j PSUM score tiles; heads share
                # att accumulator banks (h0 -> partitions 0:64, h1 -> 64:128).
                with tc.tile_pool(name="p4w", bufs=4) as p4w, \
                     tc.tile_pool(name="p4o", bufs=3) as p4o, \
                     tc.tile_pool(name="ps_att", bufs=1, space="PSUM") as pa, \
                     tc.tile_pool(name="ps_sc", bufs=4, space="PSUM") as psc:
                    for b in range(B):
                        att_ps = [pa.tile([P, 512], F32, tag=f"att{j}",
                                          name=f"att_ps{j}")
                                  for j in range(QT)]
                        for kt in range(KT):
                            k0 = kt * P
                            jmin = k0 // 512
                            o = k0 - 512 * jmin
                            for h2 in range(2):
                                hsl = slice(h2 * HS, (h2 + 1) * HS)
                                wTe = p4w.tile([P, T], BF16, tag="wTe",
                                               bufs=6)
                                s_part = p4w.tile([P, QT], F32, tag="s_part")
                                rs = p4w.tile([P, 1], F32, tag="rs")
                                for j in range(jmin, QT):
                                    if j == jmin:
                                        c0 = j * 512 + min(o, 256)
                                    else:
                                        c0 = j * 512
                                    w = (j + 1) * 512 - c0
                                    ps = psc.tile([P, 512], F32, tag="sc")
                                    nc.tensor.matmul(
                                        ps[:, 0:w],
                                        kT[hsl, b * T + k0:b * T + k0 + P],
                                        qT[hsl, b * T + c0:b * T + (j + 1) * 512],
                                        start=True, stop=True)
                                    if j == jmin:
                                        vs = k0 - c0
                                        nc.scalar.activation(
                                            out=wTe[:, k0:(j + 1) * 512],
                                            in_=ps[:, vs:w],
                                            func=mybir.ActivationFunctionType.Exp)
                                        nc.gpsimd.affine_select(
                                            out=wTe[:, k0:k0 + P],
                                            in_=wTe[:, k0:k0 + P],
                                            compare_op=mybir.AluOpType.is_ge,
                                            fill=0.0, base=0,
                                            pattern=[[1, P]], channel_multiplier=-1)
                                        nc.vector.reduce_sum(
                                            out=s_part[:, j:j + 1],
                                            in_=wTe[:, k0:(j + 1) * 512],
                                            axis=mybir.AxisListType.X)
                                    else:
                                        nc.scalar.activation(
                                            out=wTe[:, j * 512:(j + 1) * 512],
                                            in_=ps[:, 0:512],
                                            func=mybir.ActivationFunctionType.Exp,
                                            accum_out=s_part[:, j:j + 1])
                                nc.vector.reduce_sum(out=rs,
                                                     in_=s_part[:, jmin:QT],
                                                     axis=mybir.AxisListType.X)
                                nc.vector.reciprocal(out=rs, in_=rs)
                                vp = p4w.tile([P, HS], BF16, tag="vp")
                                nc.vector.tensor_scalar_mul(
                                    out=vp, in0=v_sb[:, b * KT + kt, hsl],
                                    scalar1=rs)
                                for j in range(jmin, QT):
                                    c0 = j * 512 + (o if j == jmin else 0)
                                    nc.tensor.matmul(
                                        att_ps[j][h2 * HS:(h2 + 1) * HS,
                                                  c0 - j * 512:512],
                                        vp, wTe[:, c0:(j + 1) * 512],
                                        start=(kt == 0), stop=(kt == 4 * j + 3),
                                        tile_position=(0, h2 * HS))
                        for j in range(QT):
                            oc = p4o.tile([P, 512], BF16, tag="oc")
                            nc.vector.tensor_copy(out=oc, in_=att_ps[j])
                            nc.sync.dma_start(a2a3_in[b * QT + j], oc)
                nc.leave_named_scope("P4_attn", sc_p4[0], False)
            nc.gpsimd.collective_compute(
                "AllToAll", mybir.AluOpType.bypass,
                ins=[a2a3_in.opt()], outs=[a2a3_out.opt()], replica_groups=rg)

            # ---------- P6: Wo + residual ----------
            sc_p6 = nc.enter_named_scope("P6_wo", False)
            with tc.tile_pool(name="p6", bufs=1) as p6, \
                 tc.tile_pool(name="pffn", bufs=1) as pf, \
                 tc.tile_pool(name="ps_wo", bufs=2, space="PSUM") as pw, \
                 tc.tile_pool(name="ps_tr2", bufs=2, space="PSUM") as ptr, \
                 tc.tile_pool(name="ps_z", bufs=2, space="PSUM") as pz, \
                 tc.tile_pool(name="ps_y", bufs=2, space="PSUM") as py, \
                 tc.tile_pool(name="p7w", bufs=2) as p7w, \
                 tc.tile_pool(name="p9w", bufs=2) as p9w:
                # attTs: [c_in_i=128, c_in_chunk=8, t_local=512]
                attTs = p6.tile([P, NC_, TSH], BF16)
                for s in range(NC_):
                    nc.sync.dma_start(attTs[:, s, :], a2a3_out[s])
                for ti in range(4):
                    for cj in range(2):
                        ps = pw.tile([P, 512], F32, tag="wo")
                        for r in range(NC_):
                            nc.tensor.matmul(
                                ps,
                                attTs[:, r, ti * P:(ti + 1) * P],
                                wo_sb[:, r, cj * 512:(cj + 1) * 512],
                                start=(r == 0), stop=(r == NC_ - 1))
                        csl = slice(cj * 512, (cj + 1) * 512)
                        nc.vector.tensor_add(out=x2_all[:, ti, csl], in0=ps,
                                             in1=x_all[:, ti, csl])
                        nc.vector.tensor_add(out=x2_all[:, ti, csl],
                                             in0=x2_all[:, ti, csl],
                                             in1=bo_bc[:, csl])
                nc.leave_named_scope("P6_wo", sc_p6[0], False)

                # ---------- P7-P9 (same pool scope for overlap) ----------
                h2T_sb = pf.tile([P, CO, TSH], BF16)
                uT_sb = pf.tile([P, CO, TSH], BF16)  # [j_i, j_o, t]

                # P7: LN2 + transpose
                sc_p7 = nc.enter_named_scope("P7_ln2", False)
                for ti in range(4):
                    h2_t = p7w.tile([P, C], F32, tag="h2_t")
                    _ln_apply(nc, p7w, x2_all[:, ti, :], h2_t, eps_t)
                    for cj in range(CO):
                        ps = ptr.tile([P, P], F32, tag="tr2")
                        nc.tensor.transpose(ps, h2_t[:, cj * P:(cj + 1) * P],
                                            ident)
                        nc.vector.tensor_copy(
                            out=h2T_sb[:, cj, ti * P:(ti + 1) * P], in_=ps)
                nc.leave_named_scope("P7_ln2", sc_p7[0], False)

                # P8: FFN1 (zT = W1'^T h2T, relu+bias)
                sc_p8 = nc.enter_named_scope("P8_ffn1", False)
                for jt in range(CO):
                    ps = pz.tile([P, TSH], F32, tag="z")
                    for cj in range(CO):
                        nc.tensor.matmul(
                            ps, w1_sb[:, cj, jt * P:(jt + 1) * P],
                            h2T_sb[:, cj, :],
                            start=(cj == 0), stop=(cj == CO - 1))
                    nc.scalar.activation(
                        out=uT_sb[:, jt, :], in_=ps,
                        func=mybir.ActivationFunctionType.Relu,
                        bias=bf1_sb[:, jt:jt + 1], scale=1.0)
                nc.leave_named_scope("P8_ffn1", sc_p8[0], False)

                # P9: FFN2 + residual -> out
                sc_p9 = nc.enter_named_scope("P9_ffn2", False)
                for ti in range(4):
                    for cj in range(2):
                        ps = py.tile([P, 512], F32, tag="y")
                        for jc in range(CO):
                            nc.tensor.matmul(
                                ps, uT_sb[:, jc, ti * P:(ti + 1) * P],
                                w2_sb[:, jc, cj * 512:(cj + 1) * 512],
                                start=(jc == 0), stop=(jc == CO - 1))
                        csl = slice(cj * 512, (cj + 1) * 512)
                        o_t = p9w.tile([P, 512], F32, tag="o_t")
                        nc.vector.tensor_add(out=o_t, in0=ps,
                                             in1=x2_all[:, ti, csl])
                        nc.vector.tensor_add(out=o_t, in0=o_t,
                                             in1=bf2_bc[:, csl])
                        nc.sync.dma_start(
                            out_sh[ti * P:(ti + 1) * P, csl], o_t)
                nc.leave_named_scope("P9_ffn2", sc_p9[0], False)

    split_waits(nc)
    return nc


_NC_CACHE = None


def _get_nc():
    global _NC_CACHE
    if _NC_CACHE is None:
        _NC_CACHE = build_nc()
    return _NC_CACHE


def _prep_inputs(inputs):
    """Host-side weight folding + per-core sharding."""
    x = np.asarray(inputs["x"], np.float32)
    Wq, bq = np.asarray(inputs["Wq"], np.float32), np.asarray(inputs["bq"], np.float32)
    Wk, bk = np.asarray(inputs["Wk"], np.float32), np.asarray(inputs["bk"], np.float32)
    Wv, bv = np.asarray(inputs["Wv"], np.float32), np.asarray(inputs["bv"], np.float32)
    Wo, bo = np.asarray(inputs["Wo"], np.float32), np.asarray(inputs["bo"], np.float32)
    g1, b1 = np.asarray(inputs["g1"], np.float32), np.asarray(inputs["b1"], np.float32)
    g2, b2 = np.asarray(inputs["g2"], np.float32), np.asarray(inputs["b2"], np.float32)
    W1, bf1 = np.asarray(inputs["W1"], np.float32), np.asarray(inputs["bf1"], np.float32)
    W2, bf2 = np.asarray(inputs["W2"], np.float32), np.asarray(inputs["bf2"], np.float32)

    scale = float(HS) ** -0.5
    xf = x.reshape(B * T, C)
    # folded FFN1: h2@W1+bf1 with h2 = ln*g2+b2 -> ln @ (g2*W1) + (b2@W1+bf1)
    w1f = (g2[:, None] * W1).astype(np.float32)
    bf1f = (b2 @ W1 + bf1).astype(np.float32)

    # per-qkv all-head weights [C, C], columns rank-major (rank, head, hs)
    Wq_f = (g1[:, None, None] * Wq.transpose(1, 0, 2).reshape(C, H, HS)
            ).reshape(C, C) * scale
    Wk_f = (g1[:, None, None] * Wk.transpose(1, 0, 2).reshape(C, H, HS)
            ).reshape(C, C)
    Wv_f = (g1[:, None, None] * Wv.transpose(1, 0, 2).reshape(C, H, HS)
            ).reshape(C, C)
    bq_f = (b1 @ Wq.transpose(1, 0, 2).reshape(C, C)
            + bq.reshape(C)) * scale
    bk_f = b1 @ Wk.transpose(1, 0, 2).reshape(C, C) + bk.reshape(C)
    bv_f = b1 @ Wv.transpose(1, 0, 2).reshape(C, C) + bv.reshape(C)
    bqkv_all = np.ascontiguousarray(
        np.stack([bq_f, bk_f, bv_f]).astype(np.float32))

    common = {
        "wk": np.ascontiguousarray(Wk_f.astype(ml_dtypes.bfloat16)),
        "wq": np.ascontiguousarray(Wq_f.astype(ml_dtypes.bfloat16)),
        "wv": np.ascontiguousarray(Wv_f.astype(ml_dtypes.bfloat16)),
        "bqkv": bqkv_all,
        "wo": np.ascontiguousarray(Wo.astype(ml_dtypes.bfloat16)),
        "bo": np.ascontiguousarray(bo),
        "w1": np.ascontiguousarray(w1f.astype(ml_dtypes.bfloat16)),
        "bf1": np.ascontiguousarray(bf1f),
        "w2": np.ascontiguousarray(W2.astype(ml_dtypes.bfloat16)),
        "bf2": np.ascontiguousarray(bf2),
    }
    in_maps = []
    for r in range(NC_):
        m = dict(common)
        m["x_sh"] = np.ascontiguousarray(xf[r * TSH:(r + 1) * TSH])
        in_maps.append(m)
    return in_maps


def run(inputs, trace=False):
    nc = _get_nc()
    in_maps = _prep_inputs(inputs)
    res = run_bass_kernel_spmd(nc, in_maps, core_ids=list(range(NC_)), trace=trace)
    out = np.concatenate([res.results[r]["out_sh"] for r in range(NC_)], axis=0)
    return out.reshape(B, T, C), res


def kernel(**inputs) -> np.ndarray:
    out, _ = run(inputs, trace=False)
    return out
